# Initial kernel scaffold
#
"""Concat cost-volume kernel for Trainium2 (8 NeuronCores, SPMD).

Builds out[b, ch, d, h, w] with:
  out[:, 0:C]   = left[b,c,h,w]   * (w >= d)
  out[:, C:2C]  = right[b,c,h,w-d]* (w >= d)

Sharding: channel-parallel. Core k handles channels [4k, 4k+4) of BOTH the
left and right halves, building the full disparity volume for its channels.
All cores run an identical program on different channel slices.

Per-core dataflow (all shapes hardcoded for B=2, C=32, H=128, W=240, D=48):
  - load left slice  [2,4,128,240] -> SBUF [128(h), 8*240]
  - load right slice into a zero-padded SBUF tile [128(h), 8*288]
    (48 zero columns in front of each plane)
  - mask tile [128, 288] = 48 zeros then 240 ones; the masked left plane for
    disparity d is left * mask[48-d : 288-d]; the shifted right plane for d is
    simply the padded tile's sliding window [48-d : 288-d].
  - stage each (b, side, c) unit's full [48, 240] x 128 volume in SBUF, then
    write it with ONE large (5.9 MB) DMA per unit (16 total).
"""

import sys

import numpy as np

for _p in ("/opt/trn_rl_repo",):
    if _p not in sys.path:
        sys.path.insert(0, _p)

import concourse.bass as bass
import concourse.tile as tile
from concourse import mybir
from concourse.bass_utils import run_bass_kernel_spmd

B, C, H, W = 2, 32, 128, 240
D = 48
N_CORES = 8
CPC = C // N_CORES  # channels per core (per side) = 4
PAD = D  # zero-pad columns = 48
WPAD = W + PAD  # 288
NBC = B * CPC  # input planes per side per core = 8

_NC_CACHE = {}


def _build_nc():
    nc = bass.Bass()
    f32 = mybir.dt.float32
    left_p = nc.declare_dram_parameter("left", [B, CPC, H, W], f32, isOutput=False)
    right_p = nc.declare_dram_parameter("right", [B, CPC, H, W], f32, isOutput=False)
    out_p = nc.declare_dram_parameter(
        "out", [B, 2 * CPC, D, H, W], f32, isOutput=True
    )

    with tile.TileContext(nc) as tc:
        with (
            tc.tile_pool(name="consts", bufs=1) as consts,
            tc.tile_pool(name="stage", bufs=3) as stagep,
        ):
            left_t = consts.tile([H, NBC * W], f32)
            right_t = consts.tile([H, NBC * WPAD], f32)
            mask_t = consts.tile([H, WPAD], f32)

            nc.vector.memset(mask_t[:, 0:PAD], 0.0)
            nc.vector.memset(mask_t[:, PAD:WPAD], 1.0)
            nc.gpsimd.memset(right_t[:, :], 0.0)

            nc.sync.dma_start(
                out=left_t[:, :].rearrange("h (k w) -> h k w", w=W),
                in_=left_p[:, :, :, :].rearrange("b c h w -> h (b c) w"),
            )
            nc.scalar.dma_start(
                out=right_t[:, :].rearrange("h (k w) -> h k w", w=WPAD)[:, :, PAD:],
                in_=right_p[:, :, :, :].rearrange("b c h w -> h (b c) w"),
            )

            for u in range(2 * NBC):
                b = u // (2 * CPC)
                side = (u % (2 * CPC)) // CPC
                c = u % CPC
                bc = b * CPC + c
                st = stagep.tile([H, D * W], f32, tag="st")
                for d in range(D):
                    dst = st[:, d * W : (d + 1) * W]
                    if side == 0:
                        nc.vector.tensor_mul(
                            dst,
                            left_t[:, bc * W : (bc + 1) * W],
                            mask_t[:, PAD - d : WPAD - d],
                        )
                    else:
                        base = bc * WPAD
                        nc.vector.tensor_copy(
                            dst, right_t[:, base + PAD - d : base + WPAD - d]
                        )
                eng = nc.sync if u % 2 == 0 else nc.scalar
                eng.dma_start(
                    out=out_p[b, side * CPC + c].rearrange("d h w -> h d w"),
                    in_=st[:, :],
                )
    return nc


def _get_nc():
    if "nc" not in _NC_CACHE:
        _NC_CACHE["nc"] = _build_nc()
    return _NC_CACHE["nc"]


def _make_in_maps(left, right):
    in_maps = []
    for k in range(N_CORES):
        sl = slice(k * CPC, (k + 1) * CPC)
        in_maps.append(
            {
                "left": np.ascontiguousarray(left[:, sl]),
                "right": np.ascontiguousarray(right[:, sl]),
            }
        )
    return in_maps


def _assemble(results):
    out = np.empty((B, 2 * C, D, H, W), dtype=np.float32)
    for k in range(N_CORES):
        o = results[k]["out"]
        out[:, k * CPC : (k + 1) * CPC] = o[:, :CPC]
        out[:, C + k * CPC : C + (k + 1) * CPC] = o[:, CPC:]
    return out


def run(left_feature, right_feature, max_disp, **spmd_kwargs):
    """Run on hardware; returns (full_output, BassKernelResults)."""
    assert int(max_disp) == D
    left = np.ascontiguousarray(np.asarray(left_feature, dtype=np.float32))
    right = np.ascontiguousarray(np.asarray(right_feature, dtype=np.float32))
    assert left.shape == (B, C, H, W) and right.shape == (B, C, H, W)
    res = run_bass_kernel_spmd(
        _get_nc(), _make_in_maps(left, right), list(range(N_CORES)), **spmd_kwargs
    )
    return _assemble(res.results), res


def kernel(left_feature, right_feature, max_disp):
    out, _ = run(left_feature, right_feature, max_disp)
    return out


# revision 6
# speedup vs baseline: 3.5726x; 3.5726x over previous
"""Concat cost-volume kernel for Trainium2 (8 NeuronCores, SPMD).

Builds out[b, ch, d, h, w] with:
  out[:, 0:C]   = left[b,c,h,w]   * (w >= d)
  out[:, C:2C]  = right[b,c,h,w-d]* (w >= d)

Sharding: channel-parallel. Core k handles channels [4k, 4k+4) of BOTH the
left and right halves, building the full disparity volume for its channels.
All cores run an identical program on different channel slices.

Per-core dataflow (all shapes hardcoded for B=2, C=32, H=128, W=240, D=48):
  - load left slice  [2,4,128,240] -> SBUF [128(h), 8*240]
  - load right slice into a zero-padded SBUF tile [128(h), 8*288]
    (48 zero columns in front of each plane)
  - mask tile [128, 288] = 48 zeros then 240 ones; the masked left plane for
    disparity d is left * mask[48-d : 288-d]; the shifted right plane for d is
    simply the padded tile's sliding window [48-d : 288-d].
  - stage each (b, side, c) unit's full [48, 240] x 128 volume in SBUF, then
    write it with ONE large (5.9 MB) DMA per unit (16 total).
"""

import sys

import numpy as np

for _p in ("/opt/trn_rl_repo",):
    if _p not in sys.path:
        sys.path.insert(0, _p)

import concourse.bass as bass
import concourse.tile as tile
from concourse import bacc, mybir
from concourse.bass_utils import run_bass_kernel_spmd

B, C, H, W = 2, 32, 128, 240
D = 48
N_CORES = 8
CPC = C // N_CORES  # channels per core (per side) = 4
PAD = D  # zero-pad columns = 48
WPAD = W + PAD  # 288
NBC = B * CPC  # input planes per side per core = 8

_NC_CACHE = {}


def _build_nc(units=None, repeat=1):
    nc = bacc.Bacc("TRN2", target_bir_lowering=False, debug=False)
    f32 = mybir.dt.float32
    left_p = nc.declare_dram_parameter("left", [B, CPC, H, W], f32, isOutput=False)
    right_p = nc.declare_dram_parameter("right", [B, CPC, H, W], f32, isOutput=False)
    out_p = nc.declare_dram_parameter(
        "out", [B, 2 * CPC, D, H, W], f32, isOutput=True
    )
    if units is None:
        units = range(2 * NBC)

    with tile.TileContext(nc) as tc:
        with (
            tc.tile_pool(name="consts", bufs=1) as consts,
            tc.tile_pool(name="stage", bufs=3) as stagep,
        ):
            left_t = consts.tile([H, NBC * W], f32)
            right_t = consts.tile([H, NBC * WPAD], f32)
            mask_t = consts.tile([H, WPAD], f32)

            nc.vector.memset(mask_t[:, 0:PAD], 0.0)
            nc.vector.memset(mask_t[:, PAD:WPAD], 1.0)
            nc.gpsimd.memset(right_t[:, :], 0.0)

            nc.sync.dma_start(
                out=left_t[:, :].rearrange("h (k w) -> h k w", w=W),
                in_=left_p[:, :, :, :].rearrange("b c h w -> h (b c) w"),
            )
            nc.scalar.dma_start(
                out=right_t[:, :].rearrange("h (k w) -> h k w", w=WPAD)[:, :, PAD:],
                in_=right_p[:, :, :, :].rearrange("b c h w -> h (b c) w"),
            )

            for _rep in range(repeat):
              for u in units:
                b = u // (2 * CPC)
                side = (u % (2 * CPC)) // CPC
                c = u % CPC
                bc = b * CPC + c
                st = stagep.tile([H, D * W], f32, tag="st")
                for d in range(D):
                    dst = st[:, d * W : (d + 1) * W]
                    if side == 0:
                        nc.vector.tensor_mul(
                            dst,
                            left_t[:, bc * W : (bc + 1) * W],
                            mask_t[:, PAD - d : WPAD - d],
                        )
                    else:
                        base = bc * WPAD
                        nc.vector.tensor_copy(
                            dst, right_t[:, base + PAD - d : base + WPAD - d]
                        )
                eng = nc.sync if u % 2 == 0 else nc.scalar
                eng.dma_start(
                    out=out_p[b, side * CPC + c].rearrange("d h w -> h d w"),
                    in_=st[:, :],
                )
    nc.compile()
    return nc


def _get_nc():
    if "nc" not in _NC_CACHE:
        _NC_CACHE["nc"] = _build_nc()
    return _NC_CACHE["nc"]


def _make_in_maps(left, right):
    in_maps = []
    for k in range(N_CORES):
        sl = slice(k * CPC, (k + 1) * CPC)
        in_maps.append(
            {
                "left": np.ascontiguousarray(left[:, sl]),
                "right": np.ascontiguousarray(right[:, sl]),
            }
        )
    return in_maps


def _assemble(results):
    out = np.empty((B, 2 * C, D, H, W), dtype=np.float32)
    for k in range(N_CORES):
        o = results[k]["out"]
        out[:, k * CPC : (k + 1) * CPC] = o[:, :CPC]
        out[:, C + k * CPC : C + (k + 1) * CPC] = o[:, CPC:]
    return out


def run(left_feature, right_feature, max_disp, **spmd_kwargs):
    """Run on hardware; returns (full_output, BassKernelResults)."""
    assert int(max_disp) == D
    left = np.ascontiguousarray(np.asarray(left_feature, dtype=np.float32))
    right = np.ascontiguousarray(np.asarray(right_feature, dtype=np.float32))
    assert left.shape == (B, C, H, W) and right.shape == (B, C, H, W)
    res = run_bass_kernel_spmd(
        _get_nc(), _make_in_maps(left, right), list(range(N_CORES)), **spmd_kwargs
    )
    return _assemble(res.results), res


def kernel(left_feature, right_feature, max_disp):
    out, _ = run(left_feature, right_feature, max_disp)
    return out
